# revision 1
# baseline (speedup 1.0000x reference)
"""Trainium2 Bass kernel for nn_DigitCapsules (dynamic-routing capsule layer).

Strategy (per spec sharding_hint): data-parallel over batch B=128 across 8
NeuronCores (16 examples each); dc_w replicated.  Inside each core:

  u[d,bb,n,o] = sum_i x[bb,n,i] * w[d,n,i,o] runs on the tensor engine via a
  host-built block-diagonal x operand: per group g of 8 consecutive n,
  lhsT = Xblk[g] [64=(nn,i), 128=(nn',bb)] (block-diagonal over nn), rhs =
  Wp[g] [64=(nn,i), 160=(d,o)], psum[(nn,bb), (d,o)] = u of 8 n's at full PE
  utilization.  Matmuls write d-strided psum so each bank holds (d, g3, o);
  drains to SBUF are contiguous.  u lives as [p=(nn,bb), f=(d, g, o)] fp16.

  Routing runs on DVE (+GPSIMD for the top d-slice) and ACT: b-updates via
  fp16 multiplies + fold tree over o; softmax-weighted sums via fp16
  multiplies + fold tree over g.  Softmax uses exact per-partition-row max
  rescaling (partials scaled by exp(M) in fp32 before the 128->16
  cross-partition ones-matmul fold), mathematically the true softmax.
"""

import contextlib

import numpy as np

import concourse.bacc as bacc
import concourse.bass as bass
import concourse.tile as tile
from concourse import mybir
from concourse.bass_utils import run_bass_kernel_spmd

F16 = mybir.dt.float16
F32 = mybir.dt.float32
AF = mybir.ActivationFunctionType

D, B, N, I, O = 10, 128, 1152, 8, 16
NCORES = 8
BB = B // NCORES      # 16
NN = 8                # n's per matmul group
G = N // NN           # 144 groups
DO = D * O            # 160
FU = D * G * O        # 23040 u elements per partition, layout (d, g, o)
GCH = 18              # groups per DMA chunk
NCH = G // GCH        # 8
DRAIN = 3             # groups per psum bank (3*160=480 f32)
DBANKS = 2            # banks per drain instruction


def _ap(t, dims, offset=0):
    base = t[:]
    return bass.AP(tensor=base.tensor, offset=base.offset + offset,
                   ap=[base.ap[0]] + [list(d) for d in dims])


def build_nc(debug=False, hwloop=0, gsplit=0):
    """gsplit: top-d slice handled by GPSIMD instead of DVE for heavy ops."""
    nc = bacc.Bacc(None, target_bir_lowering=False)

    xblk_d = nc.dram_tensor("xblk", [64, G * NN * BB], F16, kind="ExternalInput")
    wp_d = nc.dram_tensor("wp", [64, G * DO], F16, kind="ExternalInput")
    eones_d = nc.dram_tensor("eones", [128, 16], F32, kind="ExternalInput")
    e8_d = nc.dram_tensor("e8", [16, 128], F32, kind="ExternalInput")
    out_d = nc.dram_tensor("out", [D, BB, O], F32, kind="ExternalOutput")
    if debug:
        dbg_u = nc.dram_tensor("dbg_u", [128, FU], F16, kind="ExternalOutput")
        dbg_sm0 = nc.dram_tensor("dbg_sm0", [16, DO], F32, kind="ExternalOutput")
        dbg_b1 = nc.dram_tensor("dbg_b1", [128, D * G], F32, kind="ExternalOutput")
        dbg_sm1 = nc.dram_tensor("dbg_sm1", [16, DO], F32, kind="ExternalOutput")

    DV = D - gsplit       # d's on DVE
    # engine/d-slice pairs for heavy elementwise ops
    def slices():
        out = [(nc.vector, 0, DV)]
        if gsplit:
            out.append((nc.gpsimd, DV, gsplit))
        return out

    with tile.TileContext(nc) as tc:
        with (
            tc.tile_pool(name="const", bufs=1) as const,
            tc.tile_pool(name="big", bufs=1) as big,
            tc.tile_pool(name="stream", bufs=3) as stream,
            tc.tile_pool(name="pmm", bufs=2, space="PSUM") as pmm,
            tc.tile_pool(name="psm", bufs=2, space="PSUM") as psm,
        ):
            eones = const.tile([128, 16], F32)
            nc.sync.dma_start(eones[:], eones_d[:])
            e8t = const.tile([16, 128], F32)
            nc.sync.dma_start(e8t[:], e8_d[:])
            eones16 = const.tile([128, 16], F16)
            nc.scalar.copy(eones16[:], eones[:])

            u = big.tile([128, FU], F16)
            btmp = big.tile([128, FU], F16)
            fbA = big.tile([128, 11520], F16)
            fbB = big.tile([128, 5760], F16)
            ev = big.tile([128, D * G], F16)
            vrep8 = big.tile([128, DO * 8], F16)   # (d, g8, o)
            spart = big.tile([128, DO], F16)
            s0p = big.tile([128, DO], F16)
            b1 = big.tile([128, D * G], F32)
            btf = big.tile([128, D * G], F32)
            bsh = big.tile([128, D * G], F32)
            mrow = big.tile([128, 16], F32)
            zp = big.tile([128, 16], F32)
            esc = big.tile([128, 16], F32)
            sfin = big.tile([128, 176], F32)
            sm = big.tile([16, DO], F32)
            sq = big.tile([16, DO], F32)
            rr = big.tile([16, DO], F32)
            p1 = big.tile([16, DO], F32)
            rden = big.tile([16, DO], F32)
            tt = big.tile([16, DO], F32)
            vv = big.tile([16, DO], F32)
            rz = big.tile([16, 16], F32)

            nc.vector.memset(sfin[:, 160:176], 0.0)

            loop_cm = tc.For_i(0, hwloop, 1) if hwloop else contextlib.nullcontext()
            with loop_cm:
                # ---------------- phase 1: u generation ----------------
                for ch in range(NCH):
                    xch = stream.tile([64, GCH * 128], F16, tag="xch")
                    wch = stream.tile([64, GCH * DO], F16, tag="wch")
                    nc.sync.dma_start(xch[:], xblk_d[:, ch * GCH * 128:(ch + 1) * GCH * 128])
                    nc.sync.dma_start(wch[:], wp_d[:, ch * GCH * DO:(ch + 1) * GCH * DO])
                    for dr in range(GCH // (DRAIN * DBANKS)):
                        ps = pmm.tile([128, DBANKS * 512], F32, tag="ps")
                        for b in range(DBANKS):
                            for j in range(DRAIN):
                                gi = dr * DRAIN * DBANKS + b * DRAIN + j
                                # d-strided out: psum bank holds (d, g3, o)
                                nc.tensor.matmul(
                                    _ap(ps, [[DRAIN * O, D], [1, O]],
                                        offset=b * 512 + j * O),
                                    xch[:, gi * 128:(gi + 1) * 128],
                                    wch[:, gi * DO:(gi + 1) * DO],
                                )
                        g0 = ch * GCH + dr * DRAIN * DBANKS
                        src = _ap(ps, [[512, DBANKS], [DRAIN * O, D], [1, DRAIN * O]])
                        dst = _ap(u, [[DRAIN * O, DBANKS], [G * O, D], [1, DRAIN * O]],
                                  offset=g0 * O)
                        nc.scalar.copy(dst, src)

                def fold_g(src_tile, out_ap):
                    """Sum (d,g,o) over g via fp16 fold tree + final 9-reduce."""
                    for eng, d0, nd in slices():
                        eng.tensor_add(
                            _ap(fbA, [[72 * O, nd], [O, 72], [1, O]], offset=d0 * 72 * O),
                            _ap(src_tile, [[G * O, nd], [O, 72], [1, O]], offset=d0 * G * O),
                            _ap(src_tile, [[G * O, nd], [O, 72], [1, O]],
                                offset=d0 * G * O + 72 * O),
                        )
                        eng.tensor_add(
                            _ap(fbB, [[36 * O, nd], [O, 36], [1, O]], offset=d0 * 36 * O),
                            _ap(fbA, [[72 * O, nd], [O, 36], [1, O]], offset=d0 * 72 * O),
                            _ap(fbA, [[72 * O, nd], [O, 36], [1, O]],
                                offset=d0 * 72 * O + 36 * O),
                        )
                        eng.tensor_add(
                            _ap(fbA, [[18 * O, nd], [O, 18], [1, O]], offset=d0 * 18 * O),
                            _ap(fbB, [[36 * O, nd], [O, 18], [1, O]], offset=d0 * 36 * O),
                            _ap(fbB, [[36 * O, nd], [O, 18], [1, O]],
                                offset=d0 * 36 * O + 18 * O),
                        )
                        eng.tensor_add(
                            _ap(fbB, [[9 * O, nd], [O, 9], [1, O]], offset=d0 * 9 * O),
                            _ap(fbA, [[18 * O, nd], [O, 9], [1, O]], offset=d0 * 18 * O),
                            _ap(fbA, [[18 * O, nd], [O, 9], [1, O]],
                                offset=d0 * 18 * O + 9 * O),
                        )
                    with nc.allow_low_precision(reason="fp32 accumulation internally"):
                        nc.vector.reduce_sum(
                            out_ap,
                            _ap(fbB, [[9 * O, D], [1, O], [O, 9]]),
                            axis=mybir.AxisListType.X,
                        )

                # ---------------- iteration 0: s0 = mean(u) ----------------
                fold_g(u, s0p[:].rearrange("p (do) -> p do", do=DO))
                ps0 = psm.tile([16, DO], F32, tag="pfold")
                nc.tensor.matmul(ps0[:], eones16[:], s0p[:])
                nc.scalar.activation(sm[:], ps0[:], AF.Copy, scale=1.0 / float(N))

                def squash_to_v():
                    # v = s*|s|/(1+s^2)  (== reference squash, safe at s=0)
                    nc.vector.tensor_mul(sq[:], sm[:], sm[:])
                    nc.scalar.activation(rr[:], sm[:], AF.Abs)
                    nc.vector.tensor_scalar_add(p1[:], sq[:], 1.0)
                    nc.vector.reciprocal(rden[:], p1[:])
                    nc.vector.tensor_mul(tt[:], sm[:], rr[:])
                    nc.vector.tensor_mul(vv[:], tt[:], rden[:])

                def v_to_vrep8():
                    pv = psm.tile([128, DO], F32, tag="pvrep")
                    nc.tensor.matmul(pv[:], e8t[:], vv[:])
                    nc.vector.tensor_copy(
                        _ap(vrep8, [[8 * O, D], [O, 8], [1, O]]),
                        _ap(pv, [[16, D], [0, 8], [1, O]]),
                    )

                squash_to_v()
                v_to_vrep8()
                if debug:
                    nc.sync.dma_start(dbg_u[:], u[:])
                    nc.sync.dma_start(dbg_sm0[:], sm[:])

                # ---------------- routing iterations 1, 2 ----------------
                for it in (1, 2):
                    for eng, d0, nd in slices():
                        eng.tensor_mul(
                            _ap(btmp, [[G * O, nd], [8 * O, G // 8], [1, 8 * O]],
                                offset=d0 * G * O),
                            _ap(u, [[G * O, nd], [8 * O, G // 8], [1, 8 * O]],
                                offset=d0 * G * O),
                            _ap(vrep8, [[8 * O, nd], [0, G // 8], [1, 8 * O]],
                                offset=d0 * 8 * O),
                        )
                        # fold tree over o: 16 -> 8 -> 4 -> 2
                        eng.tensor_add(
                            _ap(fbA, [[G * 8, nd], [8, G], [1, 8]], offset=d0 * G * 8),
                            _ap(btmp, [[G * O, nd], [O, G], [1, 8]], offset=d0 * G * O),
                            _ap(btmp, [[G * O, nd], [O, G], [1, 8]], offset=d0 * G * O + 8),
                        )
                        eng.tensor_add(
                            _ap(fbB, [[G * 4, nd], [4, G], [1, 4]], offset=d0 * G * 4),
                            _ap(fbA, [[G * 8, nd], [8, G], [1, 4]], offset=d0 * G * 8),
                            _ap(fbA, [[G * 8, nd], [8, G], [1, 4]], offset=d0 * G * 8 + 4),
                        )
                        eng.tensor_add(
                            _ap(fbA, [[G * 2, nd], [2, G], [1, 2]], offset=d0 * G * 2),
                            _ap(fbB, [[G * 4, nd], [4, G], [1, 2]], offset=d0 * G * 4),
                            _ap(fbB, [[G * 4, nd], [4, G], [1, 2]], offset=d0 * G * 4 + 2),
                        )
                    bdst = btf if it == 2 else b1
                    nc.vector.tensor_add(
                        _ap(bdst, [[G, D], [1, G]]),
                        _ap(fbA, [[G * 2, D], [2, G]]),
                        _ap(fbA, [[G * 2, D], [2, G]], offset=1),
                    )
                    if it == 2:
                        nc.vector.tensor_add(b1[:], b1[:], btf[:])
                    # softmax with per-row max rescaling
                    nc.vector.reduce_max(
                        mrow[:, 0:D], _ap(b1, [[G, D], [1, G]]), axis=mybir.AxisListType.X
                    )
                    nc.vector.tensor_sub(
                        _ap(bsh, [[G, D], [1, G]]),
                        _ap(b1, [[G, D], [1, G]]),
                        _ap(mrow, [[1, D], [0, G]]),
                    )
                    nc.scalar.activation(ev[:], bsh[:], AF.Exp)
                    with nc.allow_low_precision(reason="Zp fp32 out"):
                        nc.vector.reduce_sum(
                            zp[:, 0:D], _ap(ev, [[G, D], [1, G]]), axis=mybir.AxisListType.X
                        )
                    nc.scalar.activation(esc[:, 0:D], mrow[:, 0:D], AF.Exp)
                    # stmp = u * e (broadcast over o); reuse btmp
                    for eng, d0, nd in slices():
                        eng.tensor_mul(
                            _ap(btmp, [[G * O, nd], [O, G], [1, O]], offset=d0 * G * O),
                            _ap(u, [[G * O, nd], [O, G], [1, O]], offset=d0 * G * O),
                            _ap(ev, [[G, nd], [1, G], [0, O]], offset=d0 * G),
                        )
                    fold_g(btmp, spart[:].rearrange("p (do) -> p do", do=DO))
                    nc.vector.tensor_mul(
                        _ap(sfin, [[16, D], [1, O]]),
                        _ap(spart, [[16, D], [1, O]]),
                        _ap(esc, [[1, D], [0, O]]),
                    )
                    nc.vector.tensor_mul(sfin[:, 160:160 + D], zp[:, 0:D], esc[:, 0:D])
                    pf = psm.tile([16, 176], F32, tag="pfold")
                    nc.tensor.matmul(pf[:], eones[:], sfin[:])
                    nc.vector.reciprocal(rz[:, 0:D], pf[:, 160:160 + D])
                    nc.vector.tensor_mul(
                        _ap(sm, [[16, D], [1, O]]),
                        _ap(pf, [[16, D], [1, O]]),
                        _ap(rz, [[1, D], [0, O]]),
                    )
                    squash_to_v()
                    if debug and it == 1:
                        nc.sync.dma_start(dbg_b1[:], b1[:])
                        nc.sync.dma_start(dbg_sm1[:], sm[:])
                    if it != 2:
                        v_to_vrep8()

                out_ap = bass.AP(tensor=out_d.tensor if hasattr(out_d, "tensor") else out_d,
                                 offset=0, ap=[[O, BB], [BB * O, D], [1, O]])
                nc.sync.dma_start(out_ap, vv[:])

    nc.compile()
    return nc


_NC_CACHE = None


def _get_nc():
    global _NC_CACHE
    if _NC_CACHE is None:
        _NC_CACHE = build_nc()
    return _NC_CACHE


def host_prep(x, dc_w):
    x = np.asarray(x, np.float32)
    dc_w = np.asarray(dc_w, np.float32)
    wr = dc_w.reshape(D, G, NN, I, O).transpose(2, 3, 1, 0, 4)   # [nn,i,g,d,o]
    wp = np.ascontiguousarray(wr.reshape(64, G * DO)).astype(np.float16)
    xblks = []
    for c in range(NCORES):
        xr = x[c * BB:(c + 1) * BB].reshape(BB, G, NN, I)
        blk = np.zeros((NN, I, G, NN, BB), np.float32)
        for nn in range(NN):
            blk[nn, :, :, nn, :] = xr[:, :, nn, :].transpose(2, 1, 0)
        xblks.append(np.ascontiguousarray(blk.reshape(64, G * NN * BB)).astype(np.float16))
    eones = np.zeros((128, 16), np.float32)
    for nn in range(NN):
        for bb in range(BB):
            eones[nn * BB + bb, bb] = 1.0
    e8 = np.ascontiguousarray(eones.T)
    return wp, xblks, eones, e8


def run(x, dc_w, **spmd_kwargs):
    wp, xblks, eones, e8 = host_prep(x, dc_w)
    nc = _get_nc()
    in_maps = [
        {"xblk": xblks[c], "wp": wp, "eones": eones, "e8": e8}
        for c in range(NCORES)
    ]
    res = run_bass_kernel_spmd(nc, in_maps, core_ids=list(range(NCORES)), **spmd_kwargs)
    out = np.zeros((D, B, 1, 1, O), np.float32)
    for c in range(NCORES):
        out[:, c * BB:(c + 1) * BB, 0, 0, :] = res.results[c]["out"]
    return out, res


def kernel(x, dc_w):
    return run(x, dc_w)[0]

